# revision 35
# baseline (speedup 1.0000x reference)
"""Trainium2 Bass kernel for a CQT (constant-Q transform) nn.Module.

Reference computation (per batch sample b, channel c):
    out[b, c, k, f, 0] = sum_t x[b, c, f*HOP + t] * w_re[k, t]
    out[b, c, k, f, 1] = sum_t x[b, c, f*HOP + t] * w_im[k, t]
where w_re/w_im are Hann-windowed complex exponentials with per-bin ragged
lengths (longest 11340 samples), HOP=512, 84 bins, 409 frames.

Strategy: data-parallel over the batch (1 sample per NeuronCore, 8 cores).
Per core the correlation is a banded matmul: the contraction axis t is split
into 89 chunks of 128; chunk c needs x samples x[(f + c//4)*512 + (c%4)*128 + r].
The signal is laid out once in SBUF as Xt[r, ch, rc, m] = x[ch, m*512+rc*128+r]
so every chunk's moving operand is a 410-column slice of a resident tile
(410 = 409 frames padded to the even count fp32r requires).

Weight rows are interleaved (re_k, im_k) pairs sorted by descending window
length, so the rows active in a chunk are always a prefix. Rows 0..127
(bins 0..63) form accumulation group G1 (89 chunks); rows 128..167
(bins 64..83, windows <= 281 samples) form group G2 (3 chunks). Weights are
stored column-compacted (only active rows per chunk): 1.2 MB of DMA instead
of 7.9 MB, with no change in matmul cost.

Mixed precision + PE column tiling: chunks 0..14 (>64 active rows) run
serially in float32r (full fp32 data at 1 cycle/row). The ragged tail is
where most chunks live but few rows are active, so those matmuls are packed
into disjoint column strips of the 128x128 PE array with `tile_position` and
run concurrently: chunks 15..35 (<=64 rows) two per pass, chunks 36..88
(<=32 rows) four per pass, each strip accumulating into its own PSUM band;
a cheap DVE reduction folds the bands at the end. fp32r forbids nonzero PSUM
start partitions, so the tail runs in fp16 (same 1 cycle/row; windows and
white-noise signal are well inside fp16 range). Measured end-to-end relative
error vs the fp32 reference: ~2.3e-4.
"""

import math
import os as _os
from contextlib import ExitStack

import numpy as np

import concourse.bass as bass
import concourse.mybir as mybir
import concourse.tile as tile
from concourse import bacc
from concourse.bass_utils import run_bass_kernel_spmd

# ---- problem constants (hardcoded CQT spec) ----
SR = 22050
N_BINS = 84
BPO = 12
FMIN = 32.7
HOP = 512
B, C, T = 8, 2, 220500
N_CORES = 8

LMAX = 11340           # longest window
F = 409                # frames: 1 + (T - LMAX)//HOP
NCHUNK = 89            # ceil(LMAX/128) contraction chunks
MBLK = 432             # 512-sample blocks of x: (F-1)+(NCHUNK-1)//4+1 = 431, +1 pad
FP = 410               # fp32r needs an even moving free dim; frame 409 is junk
NROWS = 2 * N_BINS     # interleaved (re, im) weight rows
G1ROWS = 128           # group 1 = rows 0..127  (bins 0..63)
G2ROWS = NROWS - G1ROWS  # 40 rows (bins 64..83)
C2S = 14               # first chunk with <=64 active rows (2-way col tiling)
C4S = 36               # first chunk with <=32 active rows (4-way col tiling)
N_WARM = int(_os.environ.get("K_NWARM", "5"))  # PE warm-up matmuls

MM_DT = mybir.dt.float32r   # head matmul dtype (full-rate fp32)
TL_DT = mybir.dt.float16    # tail matmul dtype (col-tiling legal, 1 cyc/row)

_PREP = None
_NC = None
LAST_RESULTS = None


def _params():
    """Host-side constants: compacted weight arrays + chunk geometry."""
    global _PREP
    if _PREP is not None:
        return _PREP

    Q = 1.0 / (2.0 ** (1.0 / BPO) - 1.0)
    freqs = FMIN * 2.0 ** (np.arange(N_BINS, dtype=np.float64) / BPO)
    lengths = np.round(Q * SR / freqs).astype(np.int64)
    assert int(lengths.max()) == LMAX

    t = np.arange(LMAX, dtype=np.float64)
    L = lengths.astype(np.float64)[:, None]
    mask = (t[None, :] < L).astype(np.float64)
    win = 0.5 * (1.0 - np.cos(2.0 * math.pi * t[None, :] / L)) * mask
    phase = (2.0 * math.pi / SR) * freqs[:, None] * t[None, :]
    w_re = (win * np.cos(phase)).astype(np.float32)
    w_im = (-win * np.sin(phase)).astype(np.float32)

    # rows 2k / 2k+1 = re_k / im_k; zero-pad time to NCHUNK*128
    W = np.zeros((NROWS, NCHUNK * 128), dtype=np.float32)
    W[0::2, :LMAX] = w_re
    W[1::2, :LMAX] = w_im
    WT = np.ascontiguousarray(W.T)  # (NCHUNK*128, NROWS)

    n_act = np.array([(lengths > 128 * c).sum() for c in range(NCHUNK)])
    assert n_act[0] == N_BINS and n_act[-1] >= 1
    mG1 = np.minimum(G1ROWS, 2 * n_act).astype(np.int64)
    assert mG1[C2S] <= 64 and mG1[C2S - 1] > 64
    assert mG1[C4S] <= 32 and mG1[C4S - 1] > 32
    G2C = math.ceil(int(lengths[G1ROWS // 2]) / 128)  # chunks needed by bin 64
    mG2 = (2 * n_act[:G2C] - G1ROWS).astype(np.int64)
    assert mG2[0] == G2ROWS and (mG2 > 0).all()

    base = np.zeros(NCHUNK + 1, dtype=np.int64)
    base[1:] = np.cumsum(mG1)
    SG1 = int(base[-1])
    g2base = np.zeros(G2C + 1, dtype=np.int64)
    g2base[1:] = np.cumsum(mG2)
    SG2 = int(g2base[-1])

    wg1 = np.zeros((128, SG1), dtype=np.float32)
    for c in range(NCHUNK):
        wg1[:, base[c]:base[c + 1]] = WT[128 * c:128 * (c + 1), :mG1[c]]
    wg2 = np.zeros((128, SG2), dtype=np.float32)
    for c in range(G2C):
        wg2[:, g2base[c]:g2base[c + 1]] = WT[128 * c:128 * (c + 1),
                                             G1ROWS:G1ROWS + mG2[c]]

    SH = int(base[C2S])          # head (fp32r) weight columns
    wg1h = np.ascontiguousarray(wg1[:, :SH])
    wg1t = np.ascontiguousarray(wg1[:, SH:]).astype(np.float16)  # tail, fp16

    _PREP = dict(mG1=mG1, mG2=mG2, G2C=G2C, base=base, g2base=g2base,
                 SH=SH, ST=SG1 - SH, SG2=SG2,
                 wg1h=wg1h, wg1t=wg1t, wg2=wg2)
    return _PREP


def _build_nc(rep=1):
    """Build the per-core Bass module. rep>1 wraps the matmul streams in a
    For_i hardware loop (benchmarking only)."""
    p = _params()
    mG1, mG2, G2C = p["mG1"], p["mG2"], p["G2C"]
    base, g2base = p["base"], p["g2base"]
    SH, ST, SG2 = p["SH"], p["ST"], p["SG2"]

    nc = bacc.Bacc(None, target_bir_lowering=False)
    xt_d = nc.dram_tensor("xt", (C, 4, 128, MBLK), MM_DT, kind="ExternalInput")
    xtb_d = nc.dram_tensor("xtb", (C, 4, 128, MBLK), TL_DT,
                           kind="ExternalInput")
    wh_d = nc.dram_tensor("wh", (128, SH), MM_DT, kind="ExternalInput")
    wt_d = nc.dram_tensor("wt", (128, ST), TL_DT, kind="ExternalInput")
    wg2_d = nc.dram_tensor("wg2", (128, SG2), MM_DT, kind="ExternalInput")
    out_d = nc.dram_tensor("out", (C, NROWS, F), mybir.dt.float32,
                           kind="ExternalOutput")

    with ExitStack() as ctx:
        tc = ctx.enter_context(tile.TileContext(nc))
        xp = ctx.enter_context(tc.tile_pool(name="xp", bufs=1))
        wp = ctx.enter_context(tc.tile_pool(name="wp", bufs=1))
        op = ctx.enter_context(tc.tile_pool(name="op", bufs=1))
        pp = ctx.enter_context(tc.tile_pool(name="pp", bufs=1, space="PSUM"))

        # PSUM: 4 banks per channel (head, G2, 2-way bands, 4-way bands)
        # full-bank width (512) so partition-sliced band APs stay bank-local
        ps1 = {ch: pp.tile([128, 512], mybir.dt.float32, name=f"ps1_{ch}",
                           tag=f"ps1_{ch}") for ch in range(C)}
        ps2 = {ch: pp.tile([128, 512], mybir.dt.float32, name=f"ps2_{ch}",
                           tag=f"ps2_{ch}") for ch in range(C)}
        pt2 = {ch: pp.tile([128, 512], mybir.dt.float32, name=f"pt2_{ch}",
                           tag=f"pt2_{ch}") for ch in range(C)}
        pt4 = {ch: pp.tile([128, 512], mybir.dt.float32, name=f"pt4_{ch}",
                           tag=f"pt4_{ch}") for ch in range(C)}

        # PE warm-up: dummy matmuls on a memset scratch tile bridge the cold
        # pstate while input DMAs run; target ch1's 4-way bank, whose real
        # use starts much later with start=True (pending-zero overwrites).
        warm_sb = xp.tile([128, 128], MM_DT, name="warm_sb", tag="warm_sb")
        nc.vector.memset(warm_sb[:].bitcast(mybir.dt.float32), 0.0)
        for _ in range(N_WARM):
            nc.tensor.matmul(pt4[1][:, 0:128], warm_sb[:, :], warm_sb[:, :],
                             start=True, stop=True, skip_group_check=True)

        # --- SBUF tiles + input DMA plan (two parallel queues) ---
        wh_sb = wp.tile([128, SH], MM_DT, name="wh_sb", tag="wh_sb")
        wt_sb = wp.tile([128, ST], TL_DT, name="wt_sb", tag="wt_sb")
        wg2_sb = wp.tile([128, SG2], MM_DT, name="wg2_sb", tag="wg2_sb")
        xt_sb = {(ch, rc): xp.tile([128, MBLK], MM_DT, name=f"x_{ch}_{rc}",
                                   tag=f"x_{ch}_{rc}")
                 for ch in range(C) for rc in range(4)}
        xtb_sb = {(ch, rc): xp.tile([128, MBLK], TL_DT, name=f"xb_{ch}_{rc}",
                                    tag=f"xb_{ch}_{rc}")
                  for ch in range(C) for rc in range(4)}

        # gpsimd (SWDGE) stream: weights in consumption order. Split the
        # head weights so the first matmul is gated by a single small block.
        nc.gpsimd.dma_start(wh_sb[:, 0:int(base[1])], wh_d[:, 0:int(base[1])])
        nc.gpsimd.dma_start(wh_sb[:, int(base[1]):SH],
                            wh_d[:, int(base[1]):SH])
        half_t = (ST // 2) & ~1
        nc.gpsimd.dma_start(wt_sb[:, 0:half_t], wt_d[:, 0:half_t])
        nc.gpsimd.dma_start(wt_sb[:, half_t:ST], wt_d[:, half_t:ST])
        # sync (HWDGE) stream: ch0 signal (fp32r then fp16), G2 weights,
        # then ch1 (needed only from halfway).
        nc.sync.dma_start(xt_sb[0, 0][:], xt_d[0, 0])
        nc.sync.dma_start(wg2_sb[:], wg2_d[:])
        for rc in range(1, 4):
            nc.sync.dma_start(xt_sb[0, rc][:], xt_d[0, rc])
        for rc in range(4):
            nc.sync.dma_start(xtb_sb[0, rc][:], xtb_d[0, rc])
        for rc in range(4):
            nc.sync.dma_start(xt_sb[1, rc][:], xt_d[1, rc])
        for rc in range(4):
            nc.sync.dma_start(xtb_sb[1, rc][:], xtb_d[1, rc])

        def emit_streams():
            for ch in range(C):
                # head: serial fp32r, full-width rows
                for c in range(0, C2S):
                    j, rc = divmod(c, 4)
                    m = int(mG1[c])
                    # ragged prefix accumulation: later chunks write fewer
                    # rows; start=True on chunk 0 (m=128) zero-arms the whole
                    # bank region, so the sim's group checker is skipped.
                    nc.tensor.matmul(
                        ps1[ch][0:m, 0:FP],
                        wh_sb[:, int(base[c]):int(base[c]) + m],
                        xt_sb[ch, rc][:, j:j + FP],
                        start=(c == 0), stop=(c == C2S - 1),
                        skip_group_check=True)
                # 2-way col-tiled fp16 tail: two PE column strips run
                # concurrently, each accumulating into its own 64-row band.
                for idx, c in enumerate(range(C2S, C4S)):
                    j, rc = divmod(c, 4)
                    m = int(mG1[c])
                    b = idx % 2
                    off = int(base[c]) - SH
                    nc.tensor.matmul(
                        pt2[ch][64 * b:64 * b + m, 0:FP],
                        wt_sb[:, off:off + m],
                        xtb_sb[ch, rc][:, j:j + FP],
                        start=(idx < 2), stop=(idx >= C4S - C2S - 2),
                        tile_position=(0, 64 * b),
                        skip_group_check=True)
                # 4-way col-tiled fp16 tail: 32-row bands
                n4 = NCHUNK - C4S
                for idx, c in enumerate(range(C4S, NCHUNK)):
                    j, rc = divmod(c, 4)
                    m = int(mG1[c])
                    b = idx % 4
                    off = int(base[c]) - SH
                    nc.tensor.matmul(
                        pt4[ch][32 * b:32 * b + m, 0:FP],
                        wt_sb[:, off:off + m],
                        xtb_sb[ch, rc][:, j:j + FP],
                        start=(idx < 4), stop=(idx >= n4 - 4),
                        tile_position=(0, 32 * b),
                        skip_group_check=True)
                # G2 (bins 64..83): 3 serial fp32r matmuls
                for c in range(G2C):
                    j, rc = divmod(c, 4)
                    m = int(mG2[c])
                    nc.tensor.matmul(
                        ps2[ch][0:m, 0:FP],
                        wg2_sb[:, int(g2base[c]):int(g2base[c]) + m],
                        xt_sb[ch, rc][:, j:j + FP],
                        start=(c == 0), stop=(c == G2C - 1),
                        skip_group_check=True)

        if rep > 1:
            with tc.For_i(0, rep, 1) as _i:
                emit_streams()
        else:
            emit_streams()

        # fold the tail bands into the head accumulator and write out
        for ch in range(C):
            o1 = op.tile([128, F], mybir.dt.float32, name=f"o1_{ch}",
                         tag=f"o1_{ch}")
            o2 = op.tile([G2ROWS, F], mybir.dt.float32, name=f"o2_{ch}",
                         tag=f"o2_{ch}")
            nc.vector.tensor_copy(o1[:], ps1[ch][:, 0:F])
            for b in range(2):
                m = int(mG1[C2S + b])  # rows this band ever wrote
                nc.vector.tensor_add(o1[0:m, :], o1[0:m, :],
                                     pt2[ch][64 * b:64 * b + m, 0:F])
            for b in range(4):
                m = int(mG1[C4S + b])
                nc.vector.tensor_add(o1[0:m, :], o1[0:m, :],
                                     pt4[ch][32 * b:32 * b + m, 0:F])
            nc.sync.dma_start(out_d[ch, 0:G1ROWS, :], o1[:])
            nc.vector.tensor_copy(o2[:], ps2[ch][0:G2ROWS, 0:F])
            nc.sync.dma_start(out_d[ch, G1ROWS:NROWS, :], o2[:])
    nc.finalize()
    return nc


def get_nc():
    global _NC
    if _NC is None:
        _NC = _build_nc()
    return _NC


def _pack_x(xb):
    """(C, T) -> (C, 4, 128, MBLK) with xt[ch, rc, r, m] = x[ch, m*512+rc*128+r]."""
    xpad = np.zeros((C, MBLK * 512), dtype=np.float32)
    xpad[:, :T] = xb
    return np.ascontiguousarray(
        xpad.reshape(C, MBLK, 4, 128).transpose(0, 2, 3, 1))


def kernel(x):
    global LAST_RESULTS
    x = np.asarray(x, dtype=np.float32)
    assert x.shape == (B, C, T)
    p = _params()
    in_maps = []
    for b in range(B):
        xt = _pack_x(x[b])
        in_maps.append({"xt": xt, "xtb": xt.astype(np.float16),
                        "wh": p["wg1h"], "wt": p["wg1t"], "wg2": p["wg2"]})
    nc = get_nc()
    res = run_bass_kernel_spmd(nc, in_maps, core_ids=list(range(N_CORES)))
    LAST_RESULTS = res
    out = np.empty((B, C, N_BINS, F, 2), dtype=np.float32)
    for b in range(B):
        raw = np.asarray(res.results[b]["out"])  # (C, NROWS, F)
        out[b] = raw.reshape(C, N_BINS, 2, F).transpose(0, 1, 3, 2)
    return out


# revision 41
# speedup vs baseline: 1.0090x; 1.0090x over previous
"""Trainium2 Bass kernel for a CQT (constant-Q transform) nn.Module.

Reference computation (per batch sample b, channel c):
    out[b, c, k, f, 0] = sum_t x[b, c, f*HOP + t] * w_re[k, t]
    out[b, c, k, f, 1] = sum_t x[b, c, f*HOP + t] * w_im[k, t]
where w_re/w_im are Hann-windowed complex exponentials with per-bin ragged
lengths (longest 11340 samples), HOP=512, 84 bins, 409 frames.

Strategy: data-parallel over the batch (1 sample per NeuronCore, 8 cores).
Per core the correlation is a banded matmul: the contraction axis t is split
into 89 chunks of 128; chunk c needs x samples x[(f + c//4)*512 + (c%4)*128 + r].
The signal is laid out once in SBUF as Xt[r, ch, rc, m] = x[ch, m*512+rc*128+r]
so every chunk's moving operand is a 410-column slice of a resident tile
(410 = 409 frames padded to the even count fp32r requires).

Weight rows are interleaved (re_k, im_k) pairs sorted by descending window
length, so the rows active in a chunk are always a prefix. Rows 0..127
(bins 0..63) form accumulation group G1 (89 chunks); rows 128..167
(bins 64..83, windows <= 281 samples) form group G2 (3 chunks). Weights are
stored column-compacted (only active rows per chunk): 1.2 MB of DMA instead
of 7.9 MB, with no change in matmul cost.

Mixed precision + PE column tiling: chunks 0..14 (>64 active rows) run
serially in float32r (full fp32 data at 1 cycle/row). The ragged tail is
where most chunks live but few rows are active, so those matmuls are packed
into disjoint column strips of the 128x128 PE array with `tile_position` and
run concurrently: chunks 15..35 (<=64 rows) two per pass, chunks 36..88
(<=32 rows) four per pass, each strip accumulating into its own PSUM band;
a cheap DVE reduction folds the bands at the end. fp32r forbids nonzero PSUM
start partitions, so the tail runs in fp16 (same 1 cycle/row; windows and
white-noise signal are well inside fp16 range). Measured end-to-end relative
error vs the fp32 reference: ~2.3e-4.
"""

import math
import os as _os
from contextlib import ExitStack

import numpy as np

import concourse.bass as bass
import concourse.mybir as mybir
import concourse.tile as tile
from concourse import bacc
from concourse.bass_utils import run_bass_kernel_spmd

# ---- problem constants (hardcoded CQT spec) ----
SR = 22050
N_BINS = 84
BPO = 12
FMIN = 32.7
HOP = 512
B, C, T = 8, 2, 220500
N_CORES = 8

LMAX = 11340           # longest window
F = 409                # frames: 1 + (T - LMAX)//HOP
NCHUNK = 89            # ceil(LMAX/128) contraction chunks
MBLK = 432             # 512-sample blocks of x: (F-1)+(NCHUNK-1)//4+1 = 431, +1 pad
FP = 410               # fp32r needs an even moving free dim; frame 409 is junk
NROWS = 2 * N_BINS     # interleaved (re, im) weight rows
G1ROWS = 128           # group 1 = rows 0..127  (bins 0..63)
G2ROWS = NROWS - G1ROWS  # 40 rows (bins 64..83)
C2S = 14               # first chunk with <=64 active rows (2-way col tiling)
C4S = 36               # first chunk with <=32 active rows (4-way col tiling)
N_WARM = int(_os.environ.get("K_NWARM", "5"))  # PE warm-up matmuls

MM_DT = mybir.dt.float32r   # head matmul dtype (full-rate fp32)
TL_DT = mybir.dt.float16    # tail matmul dtype (col-tiling legal, 1 cyc/row)

_PREP = None
_NC = None
LAST_RESULTS = None


def _params():
    """Host-side constants: compacted weight arrays + chunk geometry."""
    global _PREP
    if _PREP is not None:
        return _PREP

    Q = 1.0 / (2.0 ** (1.0 / BPO) - 1.0)
    freqs = FMIN * 2.0 ** (np.arange(N_BINS, dtype=np.float64) / BPO)
    lengths = np.round(Q * SR / freqs).astype(np.int64)
    assert int(lengths.max()) == LMAX

    t = np.arange(LMAX, dtype=np.float64)
    L = lengths.astype(np.float64)[:, None]
    mask = (t[None, :] < L).astype(np.float64)
    win = 0.5 * (1.0 - np.cos(2.0 * math.pi * t[None, :] / L)) * mask
    phase = (2.0 * math.pi / SR) * freqs[:, None] * t[None, :]
    w_re = (win * np.cos(phase)).astype(np.float32)
    w_im = (-win * np.sin(phase)).astype(np.float32)

    # rows 2k / 2k+1 = re_k / im_k; zero-pad time to NCHUNK*128
    W = np.zeros((NROWS, NCHUNK * 128), dtype=np.float32)
    W[0::2, :LMAX] = w_re
    W[1::2, :LMAX] = w_im
    WT = np.ascontiguousarray(W.T)  # (NCHUNK*128, NROWS)

    n_act = np.array([(lengths > 128 * c).sum() for c in range(NCHUNK)])
    assert n_act[0] == N_BINS and n_act[-1] >= 1
    mG1 = np.minimum(G1ROWS, 2 * n_act).astype(np.int64)
    assert mG1[C2S] <= 64 and mG1[C2S - 1] > 64
    assert mG1[C4S] <= 32 and mG1[C4S - 1] > 32
    G2C = math.ceil(int(lengths[G1ROWS // 2]) / 128)  # chunks needed by bin 64
    mG2 = (2 * n_act[:G2C] - G1ROWS).astype(np.int64)
    assert mG2[0] == G2ROWS and (mG2 > 0).all()

    base = np.zeros(NCHUNK + 1, dtype=np.int64)
    base[1:] = np.cumsum(mG1)
    SG1 = int(base[-1])
    g2base = np.zeros(G2C + 1, dtype=np.int64)
    g2base[1:] = np.cumsum(mG2)
    SG2 = int(g2base[-1])

    wg1 = np.zeros((128, SG1), dtype=np.float32)
    for c in range(NCHUNK):
        wg1[:, base[c]:base[c + 1]] = WT[128 * c:128 * (c + 1), :mG1[c]]
    wg2 = np.zeros((128, SG2), dtype=np.float32)
    for c in range(G2C):
        wg2[:, g2base[c]:g2base[c + 1]] = WT[128 * c:128 * (c + 1),
                                             G1ROWS:G1ROWS + mG2[c]]

    SH = int(base[C2S])          # head (fp32r) weight columns
    wg1h = np.ascontiguousarray(wg1[:, :SH])
    wg1t = np.ascontiguousarray(wg1[:, SH:]).astype(np.float16)  # tail, fp16

    _PREP = dict(mG1=mG1, mG2=mG2, G2C=G2C, base=base, g2base=g2base,
                 SH=SH, ST=SG1 - SH, SG2=SG2,
                 wg1h=wg1h, wg1t=wg1t, wg2=wg2)
    return _PREP


def _build_nc(rep=1):
    """Build the per-core Bass module. rep>1 wraps the matmul streams in a
    For_i hardware loop (benchmarking only)."""
    p = _params()
    mG1, mG2, G2C = p["mG1"], p["mG2"], p["G2C"]
    base, g2base = p["base"], p["g2base"]
    SH, ST, SG2 = p["SH"], p["ST"], p["SG2"]

    nc = bacc.Bacc(None, target_bir_lowering=False)
    xt_d = nc.dram_tensor("xt", (C, 4, 128, MBLK), MM_DT, kind="ExternalInput")
    xtb_d = nc.dram_tensor("xtb", (C, 4, 128, MBLK), TL_DT,
                           kind="ExternalInput")
    wh_d = nc.dram_tensor("wh", (128, SH), MM_DT, kind="ExternalInput")
    wt_d = nc.dram_tensor("wt", (128, ST), TL_DT, kind="ExternalInput")
    wg2_d = nc.dram_tensor("wg2", (128, SG2), TL_DT, kind="ExternalInput")
    out_d = nc.dram_tensor("out", (C, NROWS, F), mybir.dt.float32,
                           kind="ExternalOutput")

    with ExitStack() as ctx:
        tc = ctx.enter_context(tile.TileContext(nc))
        xp = ctx.enter_context(tc.tile_pool(name="xp", bufs=1))
        wp = ctx.enter_context(tc.tile_pool(name="wp", bufs=1))
        op = ctx.enter_context(tc.tile_pool(name="op", bufs=1))
        pp = ctx.enter_context(tc.tile_pool(name="pp", bufs=1, space="PSUM"))

        # PSUM: 4 banks per channel (head, G2, 2-way bands, 4-way bands)
        # full-bank width (512) so partition-sliced band APs stay bank-local
        ps1 = {ch: pp.tile([128, 512], mybir.dt.float32, name=f"ps1_{ch}",
                           tag=f"ps1_{ch}") for ch in range(C)}
        ps2 = {ch: pp.tile([128, 512], mybir.dt.float32, name=f"ps2_{ch}",
                           tag=f"ps2_{ch}") for ch in range(C)}
        pt2 = {ch: pp.tile([128, 512], mybir.dt.float32, name=f"pt2_{ch}",
                           tag=f"pt2_{ch}") for ch in range(C)}
        pt4 = {ch: pp.tile([128, 512], mybir.dt.float32, name=f"pt4_{ch}",
                           tag=f"pt4_{ch}") for ch in range(C)}

        # PE warm-up: dummy matmuls on a memset scratch tile bridge the cold
        # pstate while input DMAs run; target ch1's 4-way bank, whose real
        # use starts much later with start=True (pending-zero overwrites).
        warm_sb = xp.tile([128, 128], MM_DT, name="warm_sb", tag="warm_sb")
        nc.vector.memset(warm_sb[:].bitcast(mybir.dt.float32), 0.0)
        for _ in range(N_WARM):
            nc.tensor.matmul(pt4[1][:, 0:128], warm_sb[:, :], warm_sb[:, :],
                             start=True, stop=True, skip_group_check=True)

        # --- SBUF tiles + input DMA plan (two parallel queues) ---
        wh_sb = wp.tile([128, SH], MM_DT, name="wh_sb", tag="wh_sb")
        wt_sb = wp.tile([128, ST], TL_DT, name="wt_sb", tag="wt_sb")
        wg2_sb = wp.tile([128, SG2], TL_DT, name="wg2_sb", tag="wg2_sb")
        xt_sb = {(ch, rc): xp.tile([128, MBLK], MM_DT, name=f"x_{ch}_{rc}",
                                   tag=f"x_{ch}_{rc}")
                 for ch in range(C) for rc in range(4)}
        xtb_sb = {(ch, rc): xp.tile([128, MBLK], TL_DT, name=f"xb_{ch}_{rc}",
                                    tag=f"xb_{ch}_{rc}")
                  for ch in range(C) for rc in range(4)}

        # gpsimd (SWDGE) stream: weights in consumption order. Split the
        # head weights so the first matmul is gated by a single small block.
        nc.gpsimd.dma_start(wh_sb[:, 0:int(base[1])], wh_d[:, 0:int(base[1])])
        nc.gpsimd.dma_start(wh_sb[:, int(base[1]):SH],
                            wh_d[:, int(base[1]):SH])
        half_t = (ST // 2) & ~1
        nc.gpsimd.dma_start(wt_sb[:, 0:half_t], wt_d[:, 0:half_t])
        # sync (HWDGE) stream: ch0 signal (fp32r then fp16), G2 weights,
        # second half of the fp16 tail weights (hedges SWDGE bandwidth),
        # then ch1 (needed only from halfway).
        nc.sync.dma_start(xt_sb[0, 0][:], xt_d[0, 0])
        nc.sync.dma_start(wg2_sb[:], wg2_d[:])
        for rc in range(1, 4):
            nc.sync.dma_start(xt_sb[0, rc][:], xt_d[0, rc])
        for rc in range(4):
            nc.sync.dma_start(xtb_sb[0, rc][:], xtb_d[0, rc])
        nc.sync.dma_start(wt_sb[:, half_t:ST], wt_d[:, half_t:ST])
        for rc in range(4):
            nc.sync.dma_start(xt_sb[1, rc][:], xt_d[1, rc])
        for rc in range(4):
            nc.sync.dma_start(xtb_sb[1, rc][:], xtb_d[1, rc])

        # --- fp16 tail band plan (per channel, identical both channels) ---
        # A band = (psum tile key, partition position): an independent strip
        # accumulator. G1-tail chunks: pt2 at pos 0/64 (2 strips each), pt4 at
        # pos 0/32/64/96 (1 strip). G2's three chunks ride along: G2c0
        # (2 strips) in a pass with a pt2@64 chunk; G2c1 (ps2@0) + G2c2
        # (ps2@64, disjoint partitions) in a pass with two pt4 partners.
        def g1_chunk(c):
            j, rc = divmod(c, 4)
            return dict(m=int(mG1[c]), off=int(base[c]) - SH, j=j, rc=rc)

        def g2_chunk(c):
            j, rc = divmod(c, 4)
            return dict(m=int(mG2[c]), off=int(g2base[c]), j=j, rc=rc, g2=True)

        tw = [g1_chunk(c) for c in range(C2S, C4S)]      # 22, M<=64
        fw = [g1_chunk(c) for c in range(C4S, NCHUNK)]   # 53, M<=32
        bands = {
            "2w0":  dict(tile="pt2", pos=0,  q=tw[1::2]),
            "2w64": dict(tile="pt2", pos=64, q=[tw[0]] + tw[2::2]),
            "g2a":  dict(tile="ps2", pos=0,  q=[g2_chunk(0), g2_chunk(1)]),
            "g2b":  dict(tile="ps2", pos=64, q=[g2_chunk(2)]),
            "4w32": dict(tile="pt4", pos=32, q=[fw[0]] + fw[2::4]),
            "4w96": dict(tile="pt4", pos=96, q=[fw[1]] + fw[3::4]),
            "4w0":  dict(tile="pt4", pos=0,  q=fw[4::4]),
            "4w64": dict(tile="pt4", pos=64, q=fw[5::4]),
        }
        # emission order: hybrid passes first, then alternating rounds
        order = (["g2a", "2w64"] + ["g2a", "g2b", "4w32", "4w96"]
                 + ["2w0", "2w64"] * 11 + ["4w0", "4w32", "4w64", "4w96"] * 13)
        BAND_ROWS = {k: (b["q"][0]["m"] if not b["q"][0].get("g2") else None)
                     for k, b in bands.items()}

        def emit_streams():
            for ch in range(C):
                # head: serial fp32r, full-width rows. Ragged prefix
                # accumulation: start=True on chunk 0 (m=128) zero-arms the
                # whole bank region; the sim group checker is skipped.
                for c in range(0, C2S):
                    j, rc = divmod(c, 4)
                    m = int(mG1[c])
                    nc.tensor.matmul(
                        ps1[ch][0:m, 0:FP],
                        wh_sb[:, int(base[c]):int(base[c]) + m],
                        xt_sb[ch, rc][:, j:j + FP],
                        start=(c == 0), stop=(c == C2S - 1),
                        skip_group_check=True)
                # fp16 col-tiled tail: walk the band queues in `order`
                tiles = {"pt2": pt2[ch], "pt4": pt4[ch], "ps2": ps2[ch]}
                iters = {k: iter(b["q"]) for k, b in bands.items()}
                remaining = {k: len(b["q"]) for k, b in bands.items()}
                first = {k: True for k in bands}
                for key in order:
                    if remaining[key] == 0:
                        continue
                    cinfo = next(iters[key])
                    remaining[key] -= 1
                    b = bands[key]
                    m, pos = cinfo["m"], b["pos"]
                    wsb = wg2_sb if cinfo.get("g2") else wt_sb
                    nc.tensor.matmul(
                        tiles[b["tile"]][pos:pos + m, 0:FP],
                        wsb[:, cinfo["off"]:cinfo["off"] + m],
                        xtb_sb[ch, cinfo["rc"]][:, cinfo["j"]:cinfo["j"] + FP],
                        start=first[key], stop=(remaining[key] == 0),
                        tile_position=(0, pos),
                        skip_group_check=True)
                    first[key] = False
                for k, r in remaining.items():
                    assert r == 0, (k, r)

        if rep > 1:
            with tc.For_i(0, rep, 1) as _i:
                emit_streams()
        else:
            emit_streams()

        # fold the tail bands into the head accumulator and write out
        for ch in range(C):
            o1 = op.tile([128, F], mybir.dt.float32, name=f"o1_{ch}",
                         tag=f"o1_{ch}")
            o2 = op.tile([G2ROWS, F], mybir.dt.float32, name=f"o2_{ch}",
                         tag=f"o2_{ch}")
            nc.vector.tensor_copy(o1[:], ps1[ch][:, 0:F])
            for key in ("2w0", "2w64", "4w0", "4w32", "4w64", "4w96"):
                m = int(BAND_ROWS[key])  # rows this band ever wrote
                pos = bands[key]["pos"]
                tl = {"pt2": pt2[ch], "pt4": pt4[ch]}[bands[key]["tile"]]
                nc.vector.tensor_add(o1[0:m, :], o1[0:m, :],
                                     tl[pos:pos + m, 0:F])
            nc.sync.dma_start(out_d[ch, 0:G1ROWS, :], o1[:])
            nc.vector.tensor_copy(o2[:], ps2[ch][0:G2ROWS, 0:F])
            m2 = int(mG2[2])
            nc.vector.tensor_add(o2[0:m2, :], o2[0:m2, :],
                                 ps2[ch][64:64 + m2, 0:F])
            nc.sync.dma_start(out_d[ch, G1ROWS:NROWS, :], o2[:])
    nc.finalize()
    return nc


def get_nc():
    global _NC
    if _NC is None:
        _NC = _build_nc()
    return _NC


def _pack_x(xb):
    """(C, T) -> (C, 4, 128, MBLK) with xt[ch, rc, r, m] = x[ch, m*512+rc*128+r]."""
    xpad = np.zeros((C, MBLK * 512), dtype=np.float32)
    xpad[:, :T] = xb
    return np.ascontiguousarray(
        xpad.reshape(C, MBLK, 4, 128).transpose(0, 2, 3, 1))


def kernel(x):
    global LAST_RESULTS
    x = np.asarray(x, dtype=np.float32)
    assert x.shape == (B, C, T)
    p = _params()
    in_maps = []
    for b in range(B):
        xt = _pack_x(x[b])
        in_maps.append({"xt": xt, "xtb": xt.astype(np.float16),
                        "wh": p["wg1h"], "wt": p["wg1t"],
                        "wg2": p["wg2"].astype(np.float16)})
    nc = get_nc()
    res = run_bass_kernel_spmd(nc, in_maps, core_ids=list(range(N_CORES)))
    LAST_RESULTS = res
    out = np.empty((B, C, N_BINS, F, 2), dtype=np.float32)
    for b in range(B):
        raw = np.asarray(res.results[b]["out"])  # (C, NROWS, F)
        out[b] = raw.reshape(C, N_BINS, 2, F).transpose(0, 1, 3, 2)
    return out
